# revision 2
# baseline (speedup 1.0000x reference)
"""Trainium2 Bass kernel for nn_AtomwiseLinear (histogram_binning), v4.

Per-core pipeline (8 NeuronCores, SPMD, no collectives):
  A) 32 trips x 4 groups x 32 buckets x 32 nodes: DMA u8 digit streams,
     Act-convert to bf16, build lo/hi/hi2 one-hots with 3 TensorTensor
     is_equal ops in DVE 2x mode (16 one-hot elems per entry), accumulate
     [8x8] per-bucket count/S histograms with T-deep PE matmul chains.
     Group dg of a trip lands at PSUM tile position 32*dg (PE quad), so one
     trip fills a [128, 256] histogram tile (8 valid rows per 32-row block)
     -> one Act copy per trip and the decode runs on 128 partitions.
  B) decode crit = 3*(count>10) + mix in bf16 on [128, 4096] tiles
     (~20us); scatter crit to DRAM in node order.
  C) 32 chunks of 4096 nodes: dense y = (x@W)/sqrt(30) in bf16 with the
     4 x 1024-col blocks stacked at PSUM tile positions 0/32/64/96
     ([128, 1024] tiles), single Act copy, crit broadcast DMA, masked
     multiply on DVE, write out.
"""

import math
import os
import sys

sys.path.insert(0, "/opt/trn_rl_repo")

import numpy as np
import ml_dtypes

from concourse import bacc, bass, mybir
import concourse.tile as tile
from concourse.bass_utils import run_bass_kernel_spmd

BF16 = ml_dtypes.bfloat16

NCORES = 8
D = 30
WINDOW = 5
DEG_THRESH = 10

MINI = bool(int(os.environ.get("KMINI", "0")))
LOG_SHARD = 11 if MINI else 17
SHARD = 1 << LOG_SHARD
B = 32                  # nodes per bucket
LO = 8                  # lhsT one-hot width (local>>2)
HI = 4                  # rhs one-hot width (local&3)
NBUCK = SHARD // B      # buckets per core
G = 32                  # buckets per group
NGRP = NBUCK // G
GN = G * B              # nodes per group (1024)
TRIPG = min(4, NGRP)    # groups per trip (PE quad restack)
NTRIP = NGRP // TRIPG
T_MIN = 2 if MINI else int(os.environ.get("KTMIN", "9"))
CH = min(8192, SHARD)   # phase-C chunk width
QC = 4                  # column blocks stacked on partitions in phase C
CH4 = CH // QC

F32 = mybir.dt.float32
BF = mybir.dt.bfloat16
U8 = mybir.dt.uint8
AL = mybir.AluOpType

KUNROLL = bool(int(os.environ.get("KUNROLL", "0")))
KWBUFS = int(os.environ.get("KWBUFS", "4"))


def _host_prep(x, W, edge_index, atom_types):
    """Shard + bucket inputs. Returns (in_maps, n_real, T)."""
    n = x.shape[0]
    e0 = np.asarray(edge_index[0], dtype=np.int64)
    e1 = np.asarray(edge_index[1], dtype=np.int64)
    t8 = np.asarray(atom_types, dtype=np.uint8)

    nodes = np.concatenate([e0, e1])
    wbit = np.concatenate([t8[e1], t8[e0]])

    # node mapping (within shard): ns = dg*(LO*NTRIP*G*HI) + lo*(NTRIP*G*HI)
    #                                 + tau*(G*HI) + b*HI + u
    # so that decode rows 32*dg+lo map to contiguous DECW node blocks.
    nbuck_total = NCORES * NBUCK
    shard_of = nodes >> LOG_SHARD
    ns = nodes & (SHARD - 1)
    u = (ns % HI).astype(np.uint8)
    b_ = (ns // HI) % G
    tau = (ns // (HI * G)) % NTRIP
    lo = ((ns // (HI * G * NTRIP)) % LO).astype(np.uint8)
    dg = ns // (HI * G * NTRIP * LO)
    grp_g = (shard_of * NGRP + tau * TRIPG + dg)      # global group
    gb = grp_g * G + b_                               # global bucket
    counts = np.bincount(gb, minlength=nbuck_total)
    T = max(T_MIN, math.ceil(counts.max() / 128))
    GT = G * T
    order = np.argsort(gb, kind="stable")
    gbs = gb[order]
    sw = wbit[order]
    lo = lo[order]
    u = u[order]
    starts = np.zeros(nbuck_total, dtype=np.int64)
    np.cumsum(counts[:-1], out=starts[1:])
    within = np.arange(nodes.shape[0], dtype=np.int64) - starts[gbs]

    streams = np.full((NCORES * NGRP, 128, 3 * GT), 255, dtype=np.uint8)
    p = within & 127
    t = within >> 7
    grp = gbs // G
    bg = gbs % G
    j = bg * T + t
    hi = u
    hi2 = np.where(sw == 1, hi, np.uint8(HI)).astype(np.uint8)
    flat = streams.reshape(-1)
    base = grp * (128 * 3 * GT) + p * (3 * GT)
    flat[base + j] = lo
    flat[base + GT + j] = hi
    flat[base + 2 * GT + j] = hi2

    iota_lo = np.ascontiguousarray(np.broadcast_to(
        np.repeat(np.arange(LO, dtype=np.float32), GT), (128, LO * GT)
    )).astype(BF16)
    iota_hi = np.ascontiguousarray(np.broadcast_to(
        np.repeat(np.arange(HI, dtype=np.float32), GT), (128, HI * GT)
    )).astype(BF16)

    wsc = (np.asarray(W, dtype=np.float32) / np.sqrt(np.float32(D))).astype(BF16)
    # d5 for [128, 1]: rows 32q+d -> d//5 for d<30, sentinel 9 for pad rows
    d5v = np.full((QC, 32), 9.0, dtype=np.float32)
    d5v[:, 0:D] = (np.arange(D, dtype=np.float32) // WINDOW)[None, :]
    d5 = d5v.reshape(QC * 32, 1).astype(BF16)
    # broadcast-const rows (per-partition, HI wide): 0, 0.5, 10.5, 3
    bc = np.ascontiguousarray(np.broadcast_to(
        np.array([0.0, 0.5, DEG_THRESH + 0.5, 3.0], dtype=np.float32)[None, :, None],
        (128, 4, HI),
    ).reshape(128, 4 * HI)).astype(BF16)

    DECW = NTRIP * G * HI

    def restack(v):
        # value per node -> [32*dg + lo, tau*(G*HI) + b*HI + u]; pad rows zero
        out = np.zeros((TRIPG, 32, DECW), dtype=np.float32)
        out[:, 0:LO] = v.reshape(TRIPG, LO, DECW)
        return np.ascontiguousarray(out.reshape(TRIPG * 32, DECW)).astype(BF16)

    in_maps = []
    n_real = []
    for c in range(NCORES):
        lo_g = c * SHARD
        hi_g = min(n, (c + 1) * SHARD)
        nc_real = max(0, hi_g - lo_g)
        n_real.append(nc_real)
        xs = np.zeros((SHARD, D), dtype=np.float32)
        ts = np.zeros(SHARD, dtype=np.float32)
        if nc_real > 0:
            xs[:nc_real] = x[lo_g:hi_g]
            ts[:nc_real] = t8[lo_g:hi_g]
        xt = np.ascontiguousarray(xs.T).astype(BF16)       # [D, SHARD]
        th2 = restack(2.0 * ts)
        invth = restack(1.0 - ts)
        in_maps.append(
            {
                "streams": streams[c * NGRP : (c + 1) * NGRP],
                "xt": xt,
                "th2": th2,
                "invth": invth,
                "wsc": wsc,
                "d5": d5,
                "bc": bc,
                "iota_lo": iota_lo,
                "iota_hi": iota_hi,
            }
        )
    return in_maps, n_real, T


def build_nc(T):
    GT = G * T
    DECW = NTRIP * G * HI          # decode tile width (4096)
    TW = G * 2 * HI                # hist cols per trip (256)
    HISTW = NTRIP * TW             # hist tile width (8192)
    PR = TRIPG * 32                # partition rows used by hist/decode tiles
    nc = bacc.Bacc("TRN2", target_bir_lowering=False, debug=False, num_devices=NCORES)
    streams_d = nc.dram_tensor("streams", [NGRP, 128, 3 * GT], U8, kind="ExternalInput")
    xt_d = nc.dram_tensor("xt", [D, SHARD], BF, kind="ExternalInput")
    th2_d = nc.dram_tensor("th2", [PR, DECW], BF, kind="ExternalInput")
    invth_d = nc.dram_tensor("invth", [PR, DECW], BF, kind="ExternalInput")
    wsc_d = nc.dram_tensor("wsc", [D, D], BF, kind="ExternalInput")
    d5_d = nc.dram_tensor("d5", [QC * 32, 1], BF, kind="ExternalInput")
    bc_d = nc.dram_tensor("bc", [128, 4 * HI], BF, kind="ExternalInput")
    iota_lo_d = nc.dram_tensor("iota_lo", [128, LO * GT], BF, kind="ExternalInput")
    iota_hi_d = nc.dram_tensor("iota_hi", [128, HI * GT], BF, kind="ExternalInput")
    outt_d = nc.dram_tensor("outt", [D, SHARD], BF, kind="ExternalOutput")
    critd = nc.dram_tensor("crit_bounce", [1, SHARD], BF)

    with tile.TileContext(nc) as tc:
        with tc.tile_pool(name="const", bufs=1) as cpool:
            iota_lo = cpool.tile([128, LO * GT], BF)
            iota_hi = cpool.tile([128, HI * GT], BF)
            wsc = cpool.tile([D, D], BF)
            d5 = cpool.tile([QC * 32, 1], BF)
            bc = cpool.tile([128, 4 * HI], BF)
            th2 = cpool.tile([PR, DECW], BF)
            invth = cpool.tile([PR, DECW], BF)
            hist = cpool.tile([PR, HISTW], BF)

            for tl, dr in [
                (iota_lo, iota_lo_d), (iota_hi, iota_hi_d), (wsc, wsc_d),
                (d5, d5_d), (bc, bc_d), (th2, th2_d), (invth, invth_d),
            ]:
                nc.sync.dma_start(out=tl[:], in_=dr[:])

            # ---- Phase A: histogram ----
            wpool = tc.alloc_tile_pool(name="work", bufs=KWBUFS)
            ppool = tc.alloc_tile_pool(name="psum", bufs=2, space="PSUM")

            def do_group(g, dg, ps):
                """g: group index (reg expr or int); dg: python int in [0,TRIPG)."""
                raw = wpool.tile([128, 3 * GT], U8, tag="raw")
                nc.sync.dma_start(out=raw[:], in_=streams_d[bass.ds(g, 1), :, :])
                conv = wpool.tile([128, 3 * GT], BF, tag="conv")
                nc.scalar.copy(out=conv[:], in_=raw[:])
                oh = wpool.tile([128, (LO + 2 * HI) * GT], BF, tag="oh")
                c3 = conv[:].rearrange("p (f j) -> p f j", j=GT)
                lo_out = oh[:, 0 : LO * GT].rearrange("p (l j) -> p l j", j=GT)
                il3 = iota_lo[:].rearrange("p (l j) -> p l j", j=GT)
                nc.vector.tensor_tensor(
                    out=lo_out, in0=il3,
                    in1=c3[:, 0:1, :].to_broadcast([128, LO, GT]),
                    op=AL.is_equal,
                )
                ih3 = iota_hi[:].rearrange("p (h j) -> p h j", j=GT)
                hi1_out = oh[:, LO * GT : (LO + HI) * GT].rearrange(
                    "p (h j) -> p h j", j=GT
                )
                nc.vector.tensor_tensor(
                    out=hi1_out, in0=ih3,
                    in1=c3[:, 1:2, :].to_broadcast([128, HI, GT]),
                    op=AL.is_equal,
                )
                hi2_out = oh[:, (LO + HI) * GT :].rearrange("p (h j) -> p h j", j=GT)
                nc.vector.tensor_tensor(
                    out=hi2_out, in0=ih3,
                    in1=c3[:, 2:3, :].to_broadcast([128, HI, GT]),
                    op=AL.is_equal,
                )
                lo3 = oh[:, 0 : LO * GT].rearrange("p (l j) -> p l j", j=GT)
                hi3 = oh[:, LO * GT :].rearrange("p (m j) -> p m j", j=GT)
                for b in range(G):
                    for t in range(T):
                        jj = b * T + t
                        nc.tensor.matmul(
                            ps[32 * dg : 32 * dg + LO,
                               b * 2 * HI : (b + 1) * 2 * HI],
                            lhsT=lo3[:, :, jj], rhs=hi3[:, :, jj],
                            start=(t == 0), stop=(t == T - 1),
                            tile_position=(0, 32 * dg),
                        )

            def do_trip(gv, tripcol):
                ps = ppool.tile([128, TW], F32, tag="ps")
                nc.scalar.memzero(ps[:])
                for dg in range(TRIPG):
                    do_group(gv + dg, dg, ps)
                nc.scalar.copy(out=hist[:, tripcol], in_=ps[0:PR, :])

            if KUNROLL:
                for tau in range(NTRIP):
                    do_trip(tau * TRIPG, bass.ds(tau * TW, TW))
            else:
                with tc.For_i(
                    0, NGRP, TRIPG, hint_engines=(mybir.EngineType.PE,)
                ) as gv:
                    do_trip(gv, bass.ds(gv * (TW // TRIPG), TW))

            ppool.release()
            wpool.release()

            # ---- Phase B: decode crit = 3*(count>10) + mix ----
            dcpool = tc.alloc_tile_pool(name="dec", bufs=1)
            hist3 = hist[:].rearrange("p (b u) -> p b u", u=2 * HI)
            cnt = hist3[:, :, 0:HI]
            aa = hist3[:, :, HI : 2 * HI]
            ta = dcpool.tile([PR, DECW], BF)
            tb = dcpool.tile([PR, DECW], BF)

            def v3(tt):
                return tt[:].rearrange("p (b u) -> p b u", u=HI)

            NB3 = NTRIP * G
            bc3 = bc[0:PR, :].rearrange("p (k u) -> p k u", u=HI)

            def bcast(k):
                return bc3[:, k : k + 1, :].to_broadcast([PR, NB3, HI])

            half_s = bc[0:PR, 1 * HI : 1 * HI + 1]
            three_s = bc[0:PR, 3 * HI : 3 * HI + 1]
            nc.vector.tensor_tensor(out=v3(ta), in0=aa, in1=bcast(0), op=AL.is_equal)
            nc.vector.tensor_tensor(out=ta[:], in0=ta[:], in1=invth[:], op=AL.mult)
            nc.vector.tensor_tensor(out=v3(tb), in0=aa, in1=cnt, op=AL.is_equal)
            nc.vector.tensor_tensor(out=tb[:], in0=tb[:], in1=th2[:], op=AL.mult)
            nc.vector.tensor_tensor(out=ta[:], in0=ta[:], in1=tb[:], op=AL.add)
            nc.vector.scalar_tensor_tensor(
                out=v3(ta), in0=cnt, scalar=half_s, in1=v3(ta),
                op0=AL.is_gt, op1=AL.mult,
            )
            nc.vector.tensor_tensor(out=v3(tb), in0=cnt, in1=bcast(2), op=AL.is_gt)
            nc.vector.scalar_tensor_tensor(
                out=tb[:], in0=tb[:], scalar=three_s, in1=ta[:],
                op0=AL.mult, op1=AL.add,
            )
            # scatter crit to DRAM in node order:
            # node = dg*(LO*DECW) + lo*DECW + c
            for dg in range(TRIPG):
                critd_v = critd[
                    0:1, dg * (LO * DECW) : (dg + 1) * (LO * DECW)
                ].rearrange("o (lo c) -> (o lo) c", lo=LO)
                nc.sync.dma_start(
                    out=critd_v, in_=tb[32 * dg : 32 * dg + LO, :]
                )
            dcpool.release()

            # ---- Phase C: dense + mask + writeback ----
            dpool = tc.alloc_tile_pool(name="dense", bufs=3)
            dppool = tc.alloc_tile_pool(name="dpsum", bufs=2, space="PSUM")
            for ch in range(SHARD // CH):
                c0 = ch * CH
                critb = dpool.tile([QC * 32, CH4], BF, tag="critb")
                for q in range(QC):
                    nc.sync.dma_start(
                        out=critb[32 * q : 32 * (q + 1), :],
                        in_=critd[0:1, c0 + q * CH4 : c0 + (q + 1) * CH4]
                        .to_broadcast([32, CH4]),
                    )
                xt_t = dpool.tile([D, CH], BF, tag="xt")
                nc.scalar.dma_start(out=xt_t[:], in_=xt_d[:, c0 : c0 + CH])  # Act queue
                yp = dppool.tile([QC * 32, CH4], F32, tag="yp")
                nc.vector.memset(yp[:], 0.0)
                for q in range(QC):
                    for h in range(CH4 // 512):
                        nc.tensor.matmul(
                            yp[32 * q : 32 * q + D, h * 512 : (h + 1) * 512],
                            lhsT=wsc[:],
                            rhs=xt_t[:, q * CH4 + h * 512 : q * CH4 + (h + 1) * 512],
                            start=True, stop=True,
                            tile_position=(0, 32 * q),
                        )
                mt = dpool.tile([QC * 32, CH4], BF, tag="mt")
                nc.vector.scalar_tensor_tensor(
                    out=mt[:], in0=critb[:], scalar=d5[:], in1=yp[:],
                    op0=AL.is_equal, op1=AL.mult,
                )
                for q in range(QC):
                    nc.sync.dma_start(
                        out=outt_d[:, c0 + q * CH4 : c0 + (q + 1) * CH4],
                        in_=mt[32 * q : 32 * q + D, :],
                    )
            dppool.release()
            dpool.release()

    nc.compile()
    return nc


def _assemble(results, n_real, dtype):
    n = sum(n_real)
    out = np.empty((n, D), dtype=dtype)
    for c in range(NCORES):
        nr = n_real[c]
        if nr == 0:
            continue
        outt = results[c]["outt"]  # [D, SHARD] bf16, natural node order
        out[c * SHARD : c * SHARD + nr] = outt[:, :nr].T.astype(dtype)
    return out


def kernel(x, W, edge_index, atom_types):
    x = np.asarray(x)
    in_maps, n_real, T = _host_prep(x, W, edge_index, atom_types)
    nc = build_nc(T)
    res = run_bass_kernel_spmd(nc, in_maps, list(range(NCORES)))
    return _assemble(res.results, n_real, np.float32)


# revision 3
# speedup vs baseline: 27.0264x; 27.0264x over previous
"""Trainium2 Bass kernel for nn_AtomwiseLinear (histogram_binning).

Fully interleaved single-loop pipeline (8 NeuronCores, SPMD):
  Per trip tau (4 groups x 32 buckets x 32 nodes = 4096 contiguous nodes):
    A) DMA u8 digit streams, Act-convert to bf16, build lo/hi/hi2 one-hots
       with 3 TensorTensor is_equal ops in DVE 2x mode (16 one-hot elems
       per entry), accumulate [8x8] per-bucket count/S histograms with
       T-deep PE matmul chains at PSUM tile positions 0/32/64/96.
    B) per-trip decode crit = 3*(count>10) + mix on [128, 128] tiles;
       scatter crit to DRAM in node order (4 small DMAs).
    C) (lagged by one trip) dense y = (x@W)/sqrt(30) in bf16 at PSUM quads
       ([128, 1024] tiles), crit broadcast DMA, masked multiply on DVE,
       write out.
  The C stage of trip tau-1 runs inside trip tau's body, so its DMA
  round-trip latency hides under the next trip's DVE-bound one-hot work.
"""

import math
import os
import sys

sys.path.insert(0, "/opt/trn_rl_repo")

import numpy as np
import ml_dtypes

from concourse import bacc, bass, mybir
import concourse.tile as tile
from concourse.bass_utils import run_bass_kernel_spmd

BF16 = ml_dtypes.bfloat16

NCORES = 8
D = 30
WINDOW = 5
DEG_THRESH = 10

MINI = bool(int(os.environ.get("KMINI", "0")))
LOG_SHARD = 11 if MINI else 17
SHARD = 1 << LOG_SHARD
B = 32                  # nodes per bucket
LO = 8                  # lhsT one-hot width
HI = 4                  # rhs one-hot width
NBUCK = SHARD // B      # buckets per core
G = 32                  # buckets per group
NGRP = NBUCK // G
GN = G * B              # nodes per group (1024)
TRIPG = min(4, NGRP)    # groups per trip (PE quad restack)
NTRIP = NGRP // TRIPG
TN = TRIPG * GN         # nodes per trip (4096)
T_MIN = 2 if MINI else int(os.environ.get("KTMIN", "9"))
QC = TRIPG              # column blocks stacked on partitions in phase C
CH4 = GN                # columns per block (1024)

F32 = mybir.dt.float32
BF = mybir.dt.bfloat16
U8 = mybir.dt.uint8
AL = mybir.AluOpType

KUNROLL = bool(int(os.environ.get("KUNROLL", "0")))
KWBUFS = int(os.environ.get("KWBUFS", "4"))


def _host_prep(x, W, edge_index, atom_types):
    """Shard + bucket inputs. Returns (in_maps, n_real, T)."""
    n = x.shape[0]
    e0 = np.asarray(edge_index[0], dtype=np.int64)
    e1 = np.asarray(edge_index[1], dtype=np.int64)
    t8 = np.asarray(atom_types, dtype=np.uint8)

    nodes = np.concatenate([e0, e1])
    wbit = np.concatenate([t8[e1], t8[e0]])

    # node mapping (within shard): ns = g*GN + lo*(B*HI... ) :
    #   g = ns>>10 (group), lo = (ns>>7)&7, b = (ns>>2)&31, u = ns&3
    nbuck_total = NCORES * NBUCK
    shard_of = nodes >> LOG_SHARD
    ns = nodes & (SHARD - 1)
    u = (ns % HI).astype(np.uint8)
    b_ = (ns // HI) % G
    lo = ((ns // (HI * G)) % LO).astype(np.uint8)
    g = ns // (HI * G * LO)
    grp_g = shard_of * NGRP + g                       # global group
    gb = grp_g * G + b_                               # global bucket
    counts = np.bincount(gb, minlength=nbuck_total)
    T = max(T_MIN, math.ceil(counts.max() / 128))
    GT = G * T
    order = np.argsort(gb, kind="stable")
    gbs = gb[order]
    sw = wbit[order]
    lo = lo[order]
    u = u[order]
    starts = np.zeros(nbuck_total, dtype=np.int64)
    np.cumsum(counts[:-1], out=starts[1:])
    within = np.arange(nodes.shape[0], dtype=np.int64) - starts[gbs]

    streams = np.full((NCORES * NGRP, 128, 3 * GT), 255, dtype=np.uint8)
    p = within & 127
    t = within >> 7
    grp = gbs // G
    bg = gbs % G
    j = bg * T + t
    hi = u
    hi2 = np.where(sw == 1, hi, np.uint8(HI)).astype(np.uint8)
    flat = streams.reshape(-1)
    base = grp * (128 * 3 * GT) + p * (3 * GT)
    flat[base + j] = lo
    flat[base + GT + j] = hi
    flat[base + 2 * GT + j] = hi2

    iota_lo = np.ascontiguousarray(np.broadcast_to(
        np.repeat(np.arange(LO, dtype=np.float32), GT), (128, LO * GT)
    )).astype(BF16)
    iota_hi = np.ascontiguousarray(np.broadcast_to(
        np.repeat(np.arange(HI, dtype=np.float32), GT), (128, HI * GT)
    )).astype(BF16)

    wsc = (np.asarray(W, dtype=np.float32) / np.sqrt(np.float32(D))).astype(BF16)
    d5v = np.full((QC, 32), 9.0, dtype=np.float32)
    d5v[:, 0:D] = (np.arange(D, dtype=np.float32) // WINDOW)[None, :]
    d5 = d5v.reshape(QC * 32, 1).astype(BF16)
    bc = np.ascontiguousarray(np.broadcast_to(
        np.array([0.0, 0.5, DEG_THRESH + 0.5, 3.0], dtype=np.float32)[None, :, None],
        (128, 4, HI),
    ).reshape(128, 4 * HI)).astype(BF16)

    GHI = G * HI  # decode cols per trip (128)

    def restack(v):
        # value per node -> [32*dg + lo, tau*GHI + b*HI + u]; pad rows zero
        vv = v.reshape(NTRIP, TRIPG, LO, GHI)
        out = np.zeros((TRIPG, 32, NTRIP, GHI), dtype=np.float32)
        out[:, 0:LO] = vv.transpose(1, 2, 0, 3)
        return np.ascontiguousarray(out.reshape(TRIPG * 32, NTRIP * GHI)).astype(BF16)

    in_maps = []
    n_real = []
    for c in range(NCORES):
        lo_g = c * SHARD
        hi_g = min(n, (c + 1) * SHARD)
        nc_real = max(0, hi_g - lo_g)
        n_real.append(nc_real)
        xs = np.zeros((SHARD, D), dtype=np.float32)
        ts = np.zeros(SHARD, dtype=np.float32)
        if nc_real > 0:
            xs[:nc_real] = x[lo_g:hi_g]
            ts[:nc_real] = t8[lo_g:hi_g]
        xt = np.ascontiguousarray(xs.T).astype(BF16)       # [D, SHARD]
        th2 = restack(2.0 * ts)
        invth = restack(1.0 - ts)
        in_maps.append(
            {
                "streams": streams[c * NGRP : (c + 1) * NGRP],
                "xt": xt,
                "th2": th2,
                "invth": invth,
                "wsc": wsc,
                "d5": d5,
                "bc": bc,
                "iota_lo": iota_lo,
                "iota_hi": iota_hi,
            }
        )
    return in_maps, n_real, T


def build_nc(T):
    GT = G * T
    GHI = G * HI                   # decode cols per trip (128)
    TW = G * 2 * HI                # hist cols per trip (256)
    DECW = NTRIP * GHI
    PR = TRIPG * 32
    nc = bacc.Bacc("TRN2", target_bir_lowering=False, debug=False, num_devices=NCORES)
    streams_d = nc.dram_tensor("streams", [NGRP, 128, 3 * GT], U8, kind="ExternalInput")
    xt_d = nc.dram_tensor("xt", [D, SHARD], BF, kind="ExternalInput")
    th2_d = nc.dram_tensor("th2", [TRIPG * 32, DECW], BF, kind="ExternalInput")
    invth_d = nc.dram_tensor("invth", [TRIPG * 32, DECW], BF, kind="ExternalInput")
    wsc_d = nc.dram_tensor("wsc", [D, D], BF, kind="ExternalInput")
    d5_d = nc.dram_tensor("d5", [QC * 32, 1], BF, kind="ExternalInput")
    bc_d = nc.dram_tensor("bc", [128, 4 * HI], BF, kind="ExternalInput")
    iota_lo_d = nc.dram_tensor("iota_lo", [128, LO * GT], BF, kind="ExternalInput")
    iota_hi_d = nc.dram_tensor("iota_hi", [128, HI * GT], BF, kind="ExternalInput")
    outt_d = nc.dram_tensor("outt", [D, SHARD], BF, kind="ExternalOutput")
    critd = nc.dram_tensor("crit_bounce", [1, SHARD], BF)

    with tile.TileContext(nc) as tc:
        with tc.tile_pool(name="const", bufs=1) as cpool:
            iota_lo = cpool.tile([128, LO * GT], BF)
            iota_hi = cpool.tile([128, HI * GT], BF)
            wsc = cpool.tile([D, D], BF)
            d5 = cpool.tile([QC * 32, 1], BF)
            bc = cpool.tile([128, 4 * HI], BF)
            th2 = cpool.tile([TRIPG * 32, DECW], BF)
            invth = cpool.tile([TRIPG * 32, DECW], BF)

            for tl, dr in [
                (iota_lo, iota_lo_d), (iota_hi, iota_hi_d), (wsc, wsc_d),
                (d5, d5_d), (bc, bc_d), (th2, th2_d), (invth, invth_d),
            ]:
                nc.sync.dma_start(out=tl[:], in_=dr[:])

            wpool = tc.alloc_tile_pool(name="work", bufs=KWBUFS)
            hpool = tc.alloc_tile_pool(name="histp", bufs=2)
            dpool = tc.alloc_tile_pool(name="dense", bufs=3)
            ppool = tc.alloc_tile_pool(name="psum", bufs=2, space="PSUM")
            dppool = tc.alloc_tile_pool(name="dpsum", bufs=2, space="PSUM")

            bc3 = bc[0:PR, :].rearrange("p (k u) -> p k u", u=HI)
            half_s = bc[0:PR, 1 * HI : 1 * HI + 1]
            three_s = bc[0:PR, 3 * HI : 3 * HI + 1]

            def do_group(g, dg, ps):
                raw = wpool.tile([128, 3 * GT], U8, tag="raw")
                nc.sync.dma_start(out=raw[:], in_=streams_d[bass.ds(g, 1), :, :])
                conv = wpool.tile([128, 3 * GT], BF, tag="conv")
                nc.scalar.copy(out=conv[:], in_=raw[:])
                oh = wpool.tile([128, (LO + 2 * HI) * GT], BF, tag="oh")
                c3 = conv[:].rearrange("p (f j) -> p f j", j=GT)
                lo_out = oh[:, 0 : LO * GT].rearrange("p (l j) -> p l j", j=GT)
                il3 = iota_lo[:].rearrange("p (l j) -> p l j", j=GT)
                nc.vector.tensor_tensor(
                    out=lo_out, in0=il3,
                    in1=c3[:, 0:1, :].to_broadcast([128, LO, GT]),
                    op=AL.is_equal,
                )
                ih3 = iota_hi[:].rearrange("p (h j) -> p h j", j=GT)
                hi1_out = oh[:, LO * GT : (LO + HI) * GT].rearrange(
                    "p (h j) -> p h j", j=GT
                )
                nc.vector.tensor_tensor(
                    out=hi1_out, in0=ih3,
                    in1=c3[:, 1:2, :].to_broadcast([128, HI, GT]),
                    op=AL.is_equal,
                )
                hi2_out = oh[:, (LO + HI) * GT :].rearrange("p (h j) -> p h j", j=GT)
                nc.vector.tensor_tensor(
                    out=hi2_out, in0=ih3,
                    in1=c3[:, 2:3, :].to_broadcast([128, HI, GT]),
                    op=AL.is_equal,
                )
                lo3 = oh[:, 0 : LO * GT].rearrange("p (l j) -> p l j", j=GT)
                hi3 = oh[:, LO * GT :].rearrange("p (m j) -> p m j", j=GT)
                for b in range(G):
                    for t in range(T):
                        jj = b * T + t
                        nc.tensor.matmul(
                            ps[32 * dg : 32 * dg + LO,
                               b * 2 * HI : (b + 1) * 2 * HI],
                            lhsT=lo3[:, :, jj], rhs=hi3[:, :, jj],
                            start=(t == 0), stop=(t == T - 1),
                            tile_position=(0, 32 * dg),
                        )

            def do_A_and_decode(gv):
                """Histogram + decode + crit scatter for trip gv//TRIPG."""
                ps = ppool.tile([128, TW], F32, tag="ps")
                nc.scalar.memzero(ps[:])
                for dg in range(TRIPG):
                    do_group(gv + dg, dg, ps)
                hh = hpool.tile([PR, TW], BF, tag="hh")
                nc.scalar.copy(out=hh[:], in_=ps[0:PR, :])
                hh3 = hh[:].rearrange("p (b m) -> p b m", m=2 * HI)
                cnt = hh3[:, :, 0:HI]
                aa = hh3[:, :, HI : 2 * HI]
                ta = hpool.tile([PR, GHI], BF, tag="ta")
                tb = hpool.tile([PR, GHI], BF, tag="tb")
                tcol = bass.ds(gv * (GHI // TRIPG), GHI)
                th2s = th2[:, tcol].rearrange("p (b u) -> p b u", u=HI)
                invths = invth[:, tcol].rearrange("p (b u) -> p b u", u=HI)

                def v3(tt):
                    return tt[:].rearrange("p (b u) -> p b u", u=HI)

                def bcast(k):
                    return bc3[:, k : k + 1, :].to_broadcast([PR, G, HI])

                nc.vector.tensor_tensor(out=v3(ta), in0=aa, in1=bcast(0), op=AL.is_equal)
                nc.vector.tensor_tensor(out=v3(ta), in0=v3(ta), in1=invths, op=AL.mult)
                nc.vector.tensor_tensor(out=v3(tb), in0=aa, in1=cnt, op=AL.is_equal)
                nc.vector.tensor_tensor(out=v3(tb), in0=v3(tb), in1=th2s, op=AL.mult)
                nc.vector.tensor_tensor(out=ta[:], in0=ta[:], in1=tb[:], op=AL.add)
                nc.vector.scalar_tensor_tensor(
                    out=v3(ta), in0=cnt, scalar=half_s, in1=v3(ta),
                    op0=AL.is_gt, op1=AL.mult,
                )
                nc.vector.tensor_tensor(out=v3(tb), in0=cnt, in1=bcast(2), op=AL.is_gt)
                nc.vector.scalar_tensor_tensor(
                    out=tb[:], in0=tb[:], scalar=three_s, in1=ta[:],
                    op0=AL.mult, op1=AL.add,
                )
                # scatter: node = (gv+dg)*GN + lo*GHI + c
                for dg in range(TRIPG):
                    critd_v = critd[
                        0:1, bass.ds((gv + dg) * GN, GN)
                    ].rearrange("o (lo c) -> (o lo) c", lo=LO)
                    nc.sync.dma_start(
                        out=critd_v, in_=tb[32 * dg : 32 * dg + LO, :]
                    )

            def do_C(gv):
                """Dense + mask + writeback for trip gv//TRIPG (lagged)."""
                c0v = gv * GN
                critb = dpool.tile([QC * 32, CH4], BF, tag="critb")
                for q in range(QC):
                    nc.sync.dma_start(
                        out=critb[32 * q : 32 * (q + 1), :],
                        in_=critd[0:1, bass.ds(c0v + q * CH4, CH4)]
                        .to_broadcast([32, CH4]),
                    )
                xt_t = dpool.tile([D, TN], BF, tag="xt")
                nc.scalar.dma_start(out=xt_t[:], in_=xt_d[:, bass.ds(c0v, TN)])
                yp = dppool.tile([QC * 32, CH4], F32, tag="yp")
                nc.scalar.memzero(yp[:])
                for q in range(QC):
                    for h in range(CH4 // 512):
                        nc.tensor.matmul(
                            yp[32 * q : 32 * q + D, h * 512 : (h + 1) * 512],
                            lhsT=wsc[:],
                            rhs=xt_t[:, q * CH4 + h * 512 : q * CH4 + (h + 1) * 512],
                            start=True, stop=True,
                            tile_position=(0, 32 * q),
                        )
                mt = dpool.tile([QC * 32, CH4], BF, tag="mt")
                nc.vector.scalar_tensor_tensor(
                    out=mt[:], in0=critb[:], scalar=d5[:], in1=yp[:],
                    op0=AL.is_equal, op1=AL.mult,
                )
                for q in range(QC):
                    nc.scalar.dma_start(
                        out=outt_d[:, bass.ds(c0v + q * CH4, CH4)],
                        in_=mt[32 * q : 32 * q + D, :],
                    )

            if KUNROLL or NTRIP < 3:
                for tau in range(NTRIP):
                    do_A_and_decode(tau * TRIPG)
                    if tau > 0:
                        do_C((tau - 1) * TRIPG)
                do_C((NTRIP - 1) * TRIPG)
            else:
                do_A_and_decode(0)
                with tc.For_i(
                    TRIPG, NGRP, TRIPG, hint_engines=(mybir.EngineType.PE,)
                ) as gv:
                    do_A_and_decode(gv)
                    do_C(gv - TRIPG)
                do_C((NTRIP - 1) * TRIPG)

            dppool.release()
            ppool.release()
            dpool.release()
            hpool.release()
            wpool.release()

    nc.compile()
    return nc


def _assemble(results, n_real, dtype):
    n = sum(n_real)
    out = np.empty((n, D), dtype=dtype)
    for c in range(NCORES):
        nr = n_real[c]
        if nr == 0:
            continue
        outt = results[c]["outt"]  # [D, SHARD] bf16, natural node order
        out[c * SHARD : c * SHARD + nr] = outt[:, :nr].T.astype(dtype)
    return out


def kernel(x, W, edge_index, atom_types):
    x = np.asarray(x)
    in_maps, n_real, T = _host_prep(x, W, edge_index, atom_types)
    nc = build_nc(T)
    res = run_bass_kernel_spmd(nc, in_maps, list(range(NCORES)))
    return _assemble(res.results, n_real, np.float32)
